# revision 14
# baseline (speedup 1.0000x reference)
"""HGAT block on 8 Trainium2 NeuronCores (Bass/Tile, SPMD node-sharded), v2.

Dense reformulation with rank-1 attention factorization:
  exp(lrelu(ax+ae)) = max(exp(ax)exp(ae), exp(ax/5)exp(ae/5))
so the dense per-head attention map Z[n,m] = S[n,m]*max(A1B1, A2B2) is built
in 3 elementwise passes from per-node columns (A) and broadcast per-edge
rows (B) -- no transcendentals over the dense map. The m-major copy of Z
(for the node-side aggregation matmul) comes from the XBAR DMA-transpose,
not PE transposes. Segment softmax is unnormalized: denominators den[m] ride
the per-head allreduce next to the eo partials, and Binv/den^2 (and the
1/heads mean) fold into the per-partition eoH scaling.

BatchNorms fold into weights: BN1/BN2 -> Wh', wsx', wse' + bias rows computed
on-device after a stats allreduce (stats packed into the he_attr allreduce);
BN3 -> W3', b3'; BN4 applied as a final affine after a small stats allreduce.
All hyperedge collectives run in bf16.

Sharding: nodes N=8192 split 1024/core; per-edge partials (he_attr, eo, den)
and BN stats are all-reduced across the 8 cores.
"""

import sys
import types

import numpy as np

N, T, DM = 8192, 32, 128
M, NNZ = 1024, 131072
EPS = 1e-5
SLOPE = 0.2
NCORES = 8
NL = N // NCORES          # 1024 local nodes per core
NT = NL // 128            # 8 node tiles
MT = M // 128             # 8 edge tiles
KT = (T * DM) // 128      # 32 k-tiles
D_IN = T * DM             # 4096
CW = 1032                 # per-head collective width: 1024 eo + 8 den

_PROGRAM = None


def _ensure_ntff_hook():
    try:
        import antenv.axon_hooks  # noqa: F401
        return
    except ImportError:
        pass
    try:
        import antenv
        from trn_agent_boot.trn_boot import _ntff_profile_via_ctypes
    except ImportError:
        return
    mod = types.ModuleType("antenv.axon_hooks")
    hook = _ntff_profile_via_ctypes("/opt/axon/libaxon_pjrt.so")
    mod.get_axon_ntff_profile_hook = lambda: hook
    mod.set_axon_ntff_profile_hook = lambda h: None
    sys.modules["antenv.axon_hooks"] = mod
    antenv.axon_hooks = mod


def build_program():
    from concourse import bacc, mybir, tile, masks

    f32 = mybir.dt.float32
    bf16 = mybir.dt.bfloat16
    AF = mybir.ActivationFunctionType
    ALU = mybir.AluOpType
    AX = mybir.AxisListType
    RG = [list(range(NCORES))]

    nc = bacc.Bacc("TRN2", target_bir_lowering=False, debug=False,
                   num_devices=NCORES)

    def din(name, shape, dt=f32):
        return nc.dram_tensor(name, list(shape), dt, kind="ExternalInput")

    xTb = din("xTb", [KT, 128, NL], bf16)
    S_nm = din("S_nm", [128, NT, M], bf16)
    W1b = din("W1b", [128, KT, DM], bf16)
    W3b = din("W3b", [DM, D_IN], bf16)
    cpack = din("cpack", [128, 64], f32)
    wpack = din("wpack", [128, 656], bf16)
    bmr = din("bmr", [1, M], f32)
    dinvb = din("dinvb", [128, NL], bf16)
    outT = nc.dram_tensor("outT", [D_IN, NL], f32, kind="ExternalOutput")

    def cc_pair(name, shape, dt):
        i = nc.dram_tensor(f"{name}_in", list(shape), dt)
        o = nc.dram_tensor(f"{name}_out", list(shape), dt,
                           addr_space="Shared")
        return i, o

    he1_in, he1_out = cc_pair("he1", [128, M + 2], bf16)
    he2_in, he2_out = cc_pair("he2", [128, M + 2], bf16)
    eo1a_in, eo1a_out = cc_pair("eo1a", [128, 2 * CW], bf16)
    eo1b_in, eo1b_out = cc_pair("eo1b", [128, 2 * CW], bf16)
    eo2_in, eo2_out = cc_pair("eo2", [128, CW], bf16)
    bn3_in, bn3_out = cc_pair("bn3", [128, 2], f32)
    bn4_in, bn4_out = cc_pair("bn4", [128, 2], f32)

    # cpack columns
    C_B1, C_B3, C_G, C_BINV4, C_BINV1 = 0, 1, 33, 41, 49
    # wpack columns
    W_WH1, W_WX1, W_WE1, W_WH2, W_WX2, W_WE2 = 0, 512, 516, 520, 648, 649

    def allreduce(dst, src):
        nc.gpsimd.collective_compute(
            "AllReduce", ALU.add, replica_groups=RG,
            ins=[src[:].opt()], outs=[dst[:].opt()])

    with tile.TileContext(nc) as tc:
        with (
            tc.tile_pool(name="const", bufs=1) as constp,
            tc.tile_pool(name="state", bufs=1) as statep,
            tc.tile_pool(name="bbc", bufs=4) as bbcp,
            tc.tile_pool(name="zn", bufs=2) as znp,
            tc.tile_pool(name="zm", bufs=3) as zmp,
            tc.tile_pool(name="xk", bufs=3) as xkp,
            tc.tile_pool(name="work", bufs=4) as workp,
            tc.tile_pool(name="ps_no", bufs=1, space="PSUM") as ps_no_p,
            tc.tile_pool(name="ps_eo", bufs=2, space="PSUM") as ps_eo_p,
            tc.tile_pool(name="ps_sm", bufs=2, space="PSUM") as ps_sm_p,
        ):
            ident = constp.tile([128, 128], bf16)
            masks.make_identity(nc, ident[:])
            ones_b = constp.tile([1, 128], bf16, tag="ones_b")
            nc.gpsimd.memset(ones_b[:], 1.0)
            ones_f = constp.tile([1, 128], f32, tag="ones_f")
            nc.gpsimd.memset(ones_f[:], 1.0)
            eps_sb = constp.tile([128, 1], f32, tag="epsc")
            nc.gpsimd.memset(eps_sb[:], EPS)

            cp = constp.tile([128, 64], f32, tag="cpack")
            nc.sync.dma_start(cp[:], cpack[:])
            wp = constp.tile([128, 656], bf16, tag="wpack")
            nc.sync.dma_start(wp[:], wpack[:])
            bm_sb = constp.tile([1, M], f32, tag="bmr")
            nc.sync.dma_start(bm_sb[:], bmr[:])
            dinv_bc = constp.tile([128, NL], bf16, tag="dinvb")
            nc.sync.dma_start(dinv_bc[:], dinvb[:])
            s_sb = constp.tile([128, NT * M], bf16, tag="snm")
            nc.sync.dma_start(s_sb[:], S_nm[:].rearrange("p n m -> p (n m)"))
            w_sb = constp.tile([128, KT * DM], bf16, tag="wslot", name="w1sb")
            nc.sync.dma_start(w_sb[:], W1b[:].rearrange("p k d -> p (k d)"))

            def s_tile(nt):
                return s_sb[:, nt * M:(nt + 1) * M]

            def gb(i):
                return cp[:, C_G + i:C_G + i + 1]

            # ---- helpers -------------------------------------------------
            def transpose_cols(src_fn, dst, n128, dt=bf16):
                for q in range(0, n128, 4):
                    w = min(4, n128 - q)
                    trq = ps_sm_p.tile([128, 512], dt, tag="sm", name="trq")
                    for k in range(w):
                        nc.tensor.matmul(trq[:, k * 128:(k + 1) * 128],
                                         src_fn(q + k), ident[:],
                                         is_transpose=True)
                    nc.vector.tensor_copy(
                        dst[:, q * 128:(q + w) * 128], trq[:, 0:w * 128])

            dump = statep.tile([128, NL], bf16, tag="dump")

            def bn_scales(sum_ap, sumsq_ap, g_ap, be_ap, count, tagp):
                sc = workp.tile([128, 1], f32, tag=f"sc{tagp}",
                                name=f"sc{tagp}", bufs=1)
                sh = workp.tile([128, 1], f32, tag=f"sh{tagp}",
                                name=f"sh{tagp}", bufs=1)
                tmp = workp.tile([128, 4], f32, tag="bnt", name=f"bnt{tagp}")
                mean, var, m2, rstd = (tmp[:, i:i + 1] for i in range(4))
                nc.scalar.mul(mean, sum_ap, 1.0 / count)
                nc.scalar.mul(var, sumsq_ap, 1.0 / count)
                nc.scalar.square(m2, mean)
                nc.vector.tensor_sub(var, var, m2)
                nc.scalar.activation(rstd, var, AF.Sqrt, bias=eps_sb[:, 0:1])
                nc.vector.reciprocal(rstd, rstd)
                nc.vector.tensor_mul(sc, g_ap, rstd)
                nc.vector.tensor_mul(sh, mean, sc)
                nc.vector.tensor_sub(sh, be_ap, sh)
                return sc, sh

            # ================= W1: h1 = lrelu(x@W1 + b1) ==================
            hT = statep.tile([128, NL], f32, tag="hT")
            hT_b = statep.tile([128, NL], bf16, tag="hTb")
            hp = ps_eo_p.tile([128, 1024], f32, tag="eo", name="w1p")
            for kt in range(KT):
                xk = xkp.tile([128, NL], bf16, tag="xk")
                nc.gpsimd.dma_start(xk[:], xTb[kt, :, :])
                for i in range(2):
                    nc.tensor.matmul(
                        hp[:, i * 512:(i + 1) * 512],
                        w_sb[:, kt * DM:(kt + 1) * DM],
                        xk[:, i * 512:(i + 1) * 512],
                        start=(kt == 0), stop=(kt == KT - 1))
            for i in range(2):
                sl = slice(i * 512, (i + 1) * 512)
                nc.scalar.activation(hT[:, sl], hp[:, sl], AF.Prelu,
                                     bias=cp[:, C_B1:C_B1 + 1], alpha=SLOPE)
            nc.vector.tensor_copy(hT_b[:], hT[:])
            # W3 load into the W slot (free after last W1 matmul)
            w3_sb = constp.tile([128, KT * DM], bf16, tag="wslot",
                                name="w3sb")
            nc.sync.dma_start(w3_sb[:], W3b[:])

            # ============ per-layer bn+he edge (stats + he matmul) ========
            def bn_he_edge(lname, he_in, he_out):
                st = workp.tile([128, 2], f32, tag="st", name=f"st{lname}")
                nc.vector.reduce_sum(st[:, 0:1], hT[:], axis=AX.X)
                nc.scalar.activation(dump[:], hT[:], AF.Square,
                                     accum_out=st[:, 1:2])
                hn_b = statep.tile([128, NT * 128], bf16, tag="hnb",
                                   name=f"hnb{lname}")
                transpose_cols(
                    lambda i: hT_b[:, i * 128:(i + 1) * 128], hn_b[:], NT)
                hep = ps_eo_p.tile([128, 1024], f32, tag="eo",
                                   name=f"hep{lname}")
                for nt in range(NT):
                    for i in range(2):
                        nc.tensor.matmul(
                            hep[:, i * 512:(i + 1) * 512],
                            hn_b[:, nt * 128:(nt + 1) * 128],
                            s_tile(nt)[:, i * 512:(i + 1) * 512],
                            start=(nt == 0), stop=(nt == NT - 1))
                hei = workp.tile([128, M + 2], bf16, tag="hei",
                                 name=f"hei{lname}", bufs=1)
                nc.scalar.copy(hei[:, 0:M], hep[:])
                nc.vector.tensor_copy(hei[:, M:M + 2], st[:])
                nc.sync.dma_start(he_in[:], hei[:])
                allreduce(he_out, he_in)

            # ============ per-layer post-AR folds =========================
            def bn_folds(lname, he_out, gi, heads, wh_sl, wx_sl, we_sl):
                HF = heads * DM
                er = statep.tile([128, M + 2], bf16, tag="her",
                                 name=f"her{lname}")
                nc.sync.dma_start(er[:], he_out[:])
                strf = workp.tile([128, 2], f32, tag="st",
                                  name=f"strf{lname}")
                nc.vector.tensor_copy(strf[:], er[:, M:M + 2])
                sc, sh = bn_scales(strf[:, 0:1], strf[:, 1:2],
                                   gb(gi), gb(gi + 1), float(N), lname)
                nc.scalar.activation(hT[:], hT[:], AF.Prelu, bias=sh[:, 0:1],
                                     scale=sc[:, 0:1], alpha=1.0)
                nc.vector.tensor_copy(hT_b[:], hT[:])
                whp = statep.tile([128, 512], bf16, tag="whp",
                                  name=f"whp{lname}")
                nc.vector.tensor_scalar(whp[:, 0:HF], wp[:, wh_sl],
                                        sc[:, 0:1], None, op0=ALU.mult)
                wxp = statep.tile([128, 4], bf16, tag="wxp",
                                  name=f"wxp{lname}")
                nc.vector.tensor_scalar(wxp[:, 0:heads], wp[:, wx_sl],
                                        sc[:, 0:1], None, op0=ALU.mult)
                wsep = statep.tile([128, 4], bf16, tag="wsep",
                                   name=f"wsep{lname}")
                nc.vector.tensor_scalar(wsep[:, 0:heads], wp[:, we_sl],
                                        sc[:, 0:1], None, op0=ALU.mult)
                sh_b = workp.tile([128, 1], bf16, tag="shb",
                                  name=f"shb{lname}")
                nc.vector.tensor_copy(sh_b[:], sh[:])
                # bias_f row = sh^T Wh ; axc row = sh^T wsx  (raw weights)
                bps = ps_sm_p.tile([128, 512], f32, tag="sm", name="bps")
                nc.tensor.matmul(bps[0:1, 0:HF], sh_b[:], wp[:, wh_sl],
                                 start=True, stop=True)
                # axc row = sh^T wsx ; aec row = sh^T wse
                aps = ps_sm_p.tile([128, 512], f32, tag="sm", name="aps")
                nc.tensor.matmul(aps[0:1, 8:8 + heads], sh_b[:],
                                 wp[:, wx_sl], start=True, stop=True)
                nc.tensor.matmul(aps[0:1, 16:16 + heads], sh_b[:],
                                 wp[:, we_sl], start=True, stop=True)
                brow = statep.tile([1, 528], f32, tag="brow",
                                   name=f"brow{lname}")
                nc.vector.tensor_copy(brow[0:1, 0:HF], bps[0:1, 0:HF])
                nc.vector.tensor_copy(brow[0:1, 512:512 + heads],
                                      aps[0:1, 8:8 + heads])
                nc.vector.tensor_copy(brow[0:1, 520:520 + heads],
                                      aps[0:1, 16:16 + heads])
                axcb = statep.tile([128, 8], f32, tag="axcb",
                                   name=f"axcb{lname}")
                nc.gpsimd.partition_broadcast(axcb[:, 0:heads],
                                              brow[0:1, 512:512 + heads])
                nc.vector.tensor_scalar(axcb[:, 4:4 + heads],
                                        axcb[:, 0:heads], 0.2, None,
                                        op0=ALU.mult)
                bbps = ps_sm_p.tile([128, 512], f32, tag="sm", name="bbps")
                nc.tensor.matmul(bbps[:, 0:HF], ones_f[:],
                                 brow[0:1, 0:HF], start=True, stop=True)
                bias_bc = statep.tile([128, 512], f32, tag="biasbc",
                                      name=f"bbc{lname}")
                nc.vector.tensor_copy(bias_bc[:, 0:HF], bbps[:, 0:HF])
                return er, whp, wxp, wsep, brow, axcb, bias_bc

            # ============ layer compute: xt/ax + A/B prep =================
            def layer_prep(lname, heads, er, whp, wxp, wsep, brow, axcb,
                           bias_bc):
                HF = heads * DM
                xta = statep.tile([128, NT * 512], bf16, tag="xta",
                                  name=f"xta{lname}")
                axn = statep.tile([128, 8 * NT], f32, tag="axn",
                                  name=f"axn{lname}")
                axn3 = axn[:].rearrange("p (h t) -> p h t", h=8)
                for nt in range(NT):
                    xps = ps_eo_p.tile([128, 1024], f32, tag="eo",
                                       name="xps")
                    axps = ps_sm_p.tile([128, 512], f32, tag="sm",
                                        name="axps")
                    nc.tensor.matmul(xps[:, 0:HF],
                                     hT_b[:, nt * 128:(nt + 1) * 128],
                                     whp[:, 0:HF], start=True, stop=True)
                    nc.tensor.matmul(axps[:, 0:heads],
                                     hT_b[:, nt * 128:(nt + 1) * 128],
                                     wxp[:, 0:heads], start=True, stop=True)
                    nc.vector.scalar_tensor_tensor(
                        xta[:, nt * 512:nt * 512 + HF], xps[:, 0:HF], 1.0,
                        bias_bc[:, 0:HF], op0=ALU.mult, op1=ALU.add)
                    nc.vector.tensor_copy(axn3[:, 0:heads, nt],
                                          axps[:, 0:heads])
                A1 = statep.tile([128, 8 * NT], f32, tag="A1",
                                 name=f"A1{lname}")
                A2 = statep.tile([128, 8 * NT], f32, tag="A2",
                                 name=f"A2{lname}")
                for h in range(heads):
                    sl = slice(h * NT, (h + 1) * NT)
                    nc.scalar.activation(A1[:, sl], axn[:, sl], AF.Exp,
                                         bias=axcb[:, h:h + 1])
                    nc.scalar.activation(A2[:, sl], axn[:, sl], AF.Exp,
                                         scale=0.2, bias=axcb[:, 4 + h:5 + h])
                ae_rows = []
                for h in range(heads):
                    aeps = ps_sm_p.tile([128, 512], f32, tag="sm",
                                        name="aeps")
                    aeps2 = ps_sm_p.tile([128, 512], f32, tag="sm",
                                         name="aeps2")
                    nc.tensor.matmul(aeps[0:1, :], wsep[:, h:h + 1],
                                     er[:, 0:512], start=True, stop=True)
                    nc.tensor.matmul(aeps2[0:1, :], wsep[:, h:h + 1],
                                     er[:, 512:M], start=True, stop=True)
                    aer = statep.tile([1, M], bf16, tag=f"aer{h}",
                                      name=f"aer{lname}{h}")
                    aecs = brow[0:1, 520 + h:521 + h]
                    nc.vector.scalar_tensor_tensor(
                        aer[0:1, 0:512], bm_sb[0:1, 0:512], aecs,
                        aeps[0:1, :], op0=ALU.mult, op1=ALU.add)
                    nc.vector.scalar_tensor_tensor(
                        aer[0:1, 512:M], bm_sb[0:1, 512:M], aecs,
                        aeps2[0:1, :], op0=ALU.mult, op1=ALU.add)
                    ae_rows.append(aer)
                return xta, A1, A2, ae_rows

            # ============ per-head map build + eoT + den ==================
            def head_maps(lname, h, hg, xta, A1, A2, ae_rows, ar_sb):
                b1r = workp.tile([1, M], bf16, tag="b1r", name=f"b1r{lname}{h}", bufs=2)
                b2r = workp.tile([1, M], bf16, tag="b2r", name=f"b2r{lname}{h}", bufs=2)
                nc.scalar.activation(b1r[:], ae_rows[h][0:1, :], AF.Exp)
                nc.scalar.activation(b2r[:], ae_rows[h][0:1, :], AF.Exp,
                                     scale=0.2)
                b1bc = bbcp.tile([128, M], bf16, tag="bbc",
                                 name=f"b1bc{lname}{h}")
                b2bc = bbcp.tile([128, M], bf16, tag="bbc",
                                 name=f"b2bc{lname}{h}")
                for (row, bc) in ((b1r, b1bc), (b2r, b2bc)):
                    bps = ps_sm_p.tile([128, 512], f32, tag="sm", name="bcp")
                    bps2 = ps_sm_p.tile([128, 512], f32, tag="sm",
                                        name="bcp2")
                    nc.tensor.matmul(bps[:], ones_b[:], row[0:1, 0:512],
                                     start=True, stop=True)
                    nc.tensor.matmul(bps2[:], ones_b[:], row[0:1, 512:M],
                                     start=True, stop=True)
                    nc.scalar.copy(bc[:, 0:512], bps[:])
                    nc.scalar.copy(bc[:, 512:M], bps2[:])
                znm = znp.tile([128, NT * M], bf16, tag="znm",
                               name=f"znm{lname}{h}")
                zmn = zmp.tile([128, MT * NL], bf16, tag="zmn",
                               name=f"zmn{lname}{h}")
                zmn3 = zmn[:].rearrange("p (mt n) -> p mt n", mt=MT)
                eop = ps_eo_p.tile([128, 1024], f32, tag="eo",
                                   name=f"eop{lname}{h}")
                for nt in range(NT):
                    zsl = znm[:, nt * M:(nt + 1) * M]
                    u = workp.tile([128, M], bf16, tag="u", name="u", bufs=2)
                    nc.gpsimd.tensor_scalar(
                        u[:], b2bc[:], A2[:, h * NT + nt:h * NT + nt + 1],
                        None, op0=ALU.mult)
                    nc.vector.scalar_tensor_tensor(
                        zsl, b1bc[:], A1[:, h * NT + nt:h * NT + nt + 1],
                        u[:], op0=ALU.mult, op1=ALU.max)
                    nc.vector.tensor_mul(zsl, zsl, s_tile(nt))
                    xsl = xta[:, nt * 512 + h * DM:nt * 512 + (h + 1) * DM]
                    for i in range(2):
                        nc.tensor.matmul(
                            eop[:, i * 512:(i + 1) * 512], xsl,
                            zsl[:, i * 512:(i + 1) * 512],
                            start=(nt == 0), stop=(nt == NT - 1))
                    nc.sync.dma_start_transpose(
                        zmn3[:, :, nt * 128:(nt + 1) * 128], zsl)
                off = hg * CW
                denf = workp.tile([128, MT], f32, tag="denf", name="denf")
                for mt in range(MT):
                    nc.vector.reduce_sum(denf[:, mt:mt + 1],
                                         zmn[:, mt * NL:(mt + 1) * NL],
                                         axis=AX.X)
                nc.vector.tensor_copy(ar_sb[:, off + 1024:off + CW], denf[:])
                nc.scalar.copy(ar_sb[:, off:off + 1024], eop[:])
                return znm, zmn

            # ============ per-head post-AR: eoH + noT =====================
            def head_no(lname, h, hg, er_eo, zmn, nop, first, last,
                        binv_col):
                off = hg * CW
                sm = workp.tile([128, MT], f32, tag="smh",
                                name=f"sm{lname}{h}")
                nc.vector.tensor_scalar(sm[:], er_eo[:, off + 1024:off + CW],
                                        1e-30, None, op0=ALU.max)
                nc.vector.reciprocal(sm[:], sm[:])
                nc.vector.tensor_mul(sm[:], sm[:], sm[:])
                nc.vector.tensor_mul(sm[:], sm[:], binv_col)
                eoh = workp.tile([128, MT * 128], bf16, tag="eoh",
                                 name=f"eoh{lname}{h}", bufs=2)
                nc.scalar.dma_start_transpose(
                    eoh[:].rearrange("p (mt c) -> p mt c", mt=MT),
                    er_eo[:, off:off + 1024])
                for mt in range(MT):
                    esl = eoh[:, mt * 128:(mt + 1) * 128]
                    nc.vector.tensor_scalar(esl, esl, sm[:, mt:mt + 1],
                                            None, op0=ALU.mult)
                    for i in range(2):
                        nc.tensor.matmul(
                            nop[:, i * 512:(i + 1) * 512], esl,
                            zmn[:, mt * NL + i * 512:mt * NL + (i + 1) * 512],
                            start=(first and mt == 0),
                            stop=(last and mt == MT - 1))

            def epilogue(lname, nop):
                nsc = workp.tile([128, NL], f32, tag="nsc",
                                 name=f"nsc{lname}", bufs=1)
                nc.vector.tensor_mul(nsc[:], nop[:], dinv_bc[:])
                nc.vector.tensor_add(hT[:], hT[:], nsc[:])
                nc.vector.tensor_copy(hT_b[:], hT[:])

            # ================== LAYER 1 (4 heads) =========================
            bn_he_edge("1", he1_in, he1_out)
            er1, whp1, wxp1, wsep1, aec1, axcb1, bias_bc1 = \
                bn_folds("1", he1_out, 0, 4,
                         slice(W_WH1, W_WH1 + 512),
                         slice(W_WX1, W_WX1 + 4), slice(W_WE1, W_WE1 + 4))
            xta1, A1_1, A2_1, ae1 = layer_prep("1", 4, er1, whp1, wxp1,
                                               wsep1, aec1, axcb1, bias_bc1)
            nop1 = ps_no_p.tile([128, NL], f32, tag="no", name="no1")
            binv4 = cp[:, C_BINV4:C_BINV4 + MT]

            ar1a = statep.tile([128, 2 * CW], bf16, tag="ar1a")
            zA = [head_maps("1", h, h, xta1, A1_1, A2_1, ae1, ar1a)
                  for h in range(2)]
            nc.sync.dma_start(eo1a_in[:], ar1a[:])
            allreduce(eo1a_out, eo1a_in)

            ar1b = statep.tile([128, 2 * CW], bf16, tag="ar1b")
            zB = [head_maps("1", h, h - 2, xta1, A1_1, A2_1, ae1, ar1b)
                  for h in range(2, 4)]
            nc.sync.dma_start(eo1b_in[:], ar1b[:])
            allreduce(eo1b_out, eo1b_in)

            er1a = statep.tile([128, 2 * CW], bf16, tag="ar1a", name="er1a")
            nc.sync.dma_start(er1a[:], eo1a_out[:])
            for h in range(2):
                head_no("1", h, h, er1a, zA[h][1], nop1,
                        first=(h == 0), last=False, binv_col=binv4)
            er1b = statep.tile([128, 2 * CW], bf16, tag="ar1b", name="er1b")
            nc.sync.dma_start(er1b[:], eo1b_out[:])
            for h in range(2, 4):
                head_no("1", h, h - 2, er1b, zB[h - 2][1], nop1,
                        first=False, last=(h == 3), binv_col=binv4)
            epilogue("1", nop1)

            # ================== LAYER 2 (1 head) ==========================
            bn_he_edge("2", he2_in, he2_out)
            er2, whp2, wxp2, wsep2, aec2, axcb2, bias_bc2 = \
                bn_folds("2", he2_out, 2, 1,
                         slice(W_WH2, W_WH2 + 128),
                         slice(W_WX2, W_WX2 + 1), slice(W_WE2, W_WE2 + 1))
            xta2, A1_2, A2_2, ae2 = layer_prep("2", 1, er2, whp2, wxp2,
                                               wsep2, aec2, axcb2, bias_bc2)
            nop2 = ps_no_p.tile([128, NL], f32, tag="no", name="no2")
            binv1 = cp[:, C_BINV1:C_BINV1 + MT]

            ar2 = statep.tile([128, CW], bf16, tag="ar2")
            z2 = head_maps("2", 0, 0, xta2, A1_2, A2_2, ae2, ar2)
            nc.sync.dma_start(eo2_in[:], ar2[:])
            allreduce(eo2_out, eo2_in)
            er2e = statep.tile([128, CW], bf16, tag="ar2", name="er2e")
            nc.sync.dma_start(er2e[:], eo2_out[:])
            head_no("2", 0, 0, er2e, z2[1], nop2,
                    first=True, last=True, binv_col=binv1)
            epilogue("2", nop2)

            # ================== BN3 (stats only) ==========================
            st3 = workp.tile([128, 2], f32, tag="st", name="st3")
            nc.vector.reduce_sum(st3[:, 0:1], hT[:], axis=AX.X)
            nc.scalar.activation(dump[:], hT[:], AF.Square,
                                 accum_out=st3[:, 1:2])
            nc.sync.dma_start(bn3_in[:], st3[:])
            allreduce(bn3_out, bn3_in)
            st3r = workp.tile([128, 2], f32, tag="st", name="st3r")
            nc.sync.dma_start(st3r[:], bn3_out[:])
            sc3, sh3 = bn_scales(st3r[:, 0:1], st3r[:, 1:2],
                                 gb(4), gb(5), float(N), "3")
            nc.vector.tensor_scalar(w3_sb[:], w3_sb[:], sc3[:, 0:1],
                                    None, op0=ALU.mult)
            shr = workp.tile([128, 2], f32, tag="shr", name="shr", bufs=1)
            nc.vector.reciprocal(shr[:, 0:1], sc3[:, 0:1])
            nc.vector.tensor_mul(shr[:, 0:1], shr[:, 0:1], sh3[:, 0:1])
            shr_b = workp.tile([128, 1], bf16, tag="shrb", name="shrb",
                               bufs=1)
            nc.vector.tensor_copy(shr_b[:], shr[:, 0:1])

            # ============ W3 pass A: v = x + lrelu(h3@W3' + b3') ==========
            vch = [znp.tile([128, NT * M], bf16, tag="znm", name=f"vch{i}")
                   for i in range(2)] + \
                  [zmp.tile([128, MT * NL], bf16, tag="zmn", name=f"vch{i}")
                   for i in range(2, 4)]

            def v_slice(jc):
                t = vch[jc // 8]
                j = jc % 8
                return t[:, j * NL:(j + 1) * NL]

            bnst = statep.tile([128, KT * 12], f32, tag="bnst")
            for jc in range(KT):
                yps = ps_eo_p.tile([128, 1024], f32, tag="eo", name="yps")
                cps = ps_sm_p.tile([128, 512], f32, tag="sm", name="cps")
                wsl = w3_sb[:, jc * 128:(jc + 1) * 128]
                for i in range(2):
                    nc.tensor.matmul(yps[:, i * 512:(i + 1) * 512], wsl,
                                     hT_b[:, i * 512:(i + 1) * 512],
                                     start=True, stop=True)
                nc.tensor.matmul(cps[:, 0:1], wsl, shr_b[:],
                                 start=True, stop=True)
                bias_sb = workp.tile([128, 1], f32, tag="b3s", name="b3s",
                                     bufs=2)
                nc.vector.tensor_add(bias_sb[:], cps[:, 0:1],
                                     cp[:, C_B3 + jc:C_B3 + jc + 1])
                vtmp = workp.tile([128, NL], bf16, tag="vtmp", name="vtmp",
                                  bufs=2)
                for i in range(2):
                    sl = slice(i * 512, (i + 1) * 512)
                    nc.scalar.activation(vtmp[:, sl], yps[:, sl], AF.Prelu,
                                         bias=bias_sb[:, 0:1], alpha=SLOPE)
                xres = xkp.tile([128, NL], bf16, tag="xk", name="xres")
                nc.gpsimd.dma_start(xres[:], xTb[jc, :, :])
                vd = v_slice(jc)
                eng = nc.vector if jc % 2 == 0 else nc.gpsimd
                eng.tensor_add(vd, vtmp[:], xres[:])
                for i in range(2):
                    nc.vector.bn_stats(
                        bnst[:, jc * 12 + i * 6:jc * 12 + (i + 1) * 6],
                        vd[:, i * 512:(i + 1) * 512])
            st4m = workp.tile([128, 2], f32, tag="st", name="st4m")
            nc.vector.bn_aggr(st4m[:], bnst[:])
            st4 = workp.tile([128, 2], f32, tag="st4", name="st4", bufs=1)
            cnt4 = float(NL * T)
            nc.vector.tensor_scalar(st4[:, 0:1], st4m[:, 0:1], cnt4, None,
                                    op0=ALU.mult)
            m2t = workp.tile([128, 1], f32, tag="m2t", name="m2t")
            nc.vector.tensor_mul(m2t[:], st4m[:, 0:1], st4m[:, 0:1])
            nc.vector.tensor_add(m2t[:], m2t[:], st4m[:, 1:2])
            nc.vector.tensor_scalar(st4[:, 1:2], m2t[:], cnt4, None,
                                    op0=ALU.mult)
            nc.sync.dma_start(bn4_in[:], st4[:])
            allreduce(bn4_out, bn4_in)
            st4r = workp.tile([128, 2], f32, tag="st", name="st4r")
            nc.sync.dma_start(st4r[:], bn4_out[:])
            sc4, sh4 = bn_scales(st4r[:, 0:1], st4r[:, 1:2],
                                 gb(6), gb(7), float(N * T), "4")

            # ============ W3 pass B: out = sc4*v + sh4 ====================
            for jc in range(KT):
                ot = workp.tile([128, NL], f32, tag="ot", name="ot", bufs=2)
                vd = v_slice(jc)
                r = jc % 3
                if r == 0:
                    nc.scalar.activation(ot[:], vd, AF.Prelu,
                                         bias=sh4[:, 0:1],
                                         scale=sc4[:, 0:1], alpha=1.0)
                else:
                    eng = nc.vector if r == 1 else nc.gpsimd
                    eng.tensor_scalar(ot[:], vd, sc4[:, 0:1], sh4[:, 0:1],
                                      op0=ALU.mult, op1=ALU.add)
                nc.gpsimd.dma_start(outT[jc * 128:(jc + 1) * 128, :], ot[:])

    nc.compile()
    return nc


def _prep_inputs(inputs):
    """Host-side preprocessing: shard, transpose, fold weights, build S."""
    x = np.ascontiguousarray(np.asarray(inputs["x"], np.float32))
    he_n = np.asarray(inputs["he_nodes"]).astype(np.int64)
    he_e = np.asarray(inputs["he_edges"]).astype(np.int64)
    W1 = np.asarray(inputs["W1"], np.float32)
    b1 = np.asarray(inputs["b1"], np.float32)
    Wh1 = np.asarray(inputs["Wh1"], np.float32)
    att1 = np.asarray(inputs["att1"], np.float32)
    Wh2 = np.asarray(inputs["Wh2"], np.float32)
    att2 = np.asarray(inputs["att2"], np.float32)
    W3 = np.asarray(inputs["W3"], np.float32)
    b3 = np.asarray(inputs["b3"], np.float32)

    try:
        import ml_dtypes
        bf = ml_dtypes.bfloat16
    except ImportError:  # pragma: no cover
        import jax.numpy as jnp
        bf = jnp.bfloat16

    S = np.zeros((M, N), np.float32)
    np.add.at(S, (he_e, he_n), 1.0)
    Dn = S.sum(axis=0)
    Bm = S.sum(axis=1)
    Dinv = np.where(Dn > 0, 1.0 / np.maximum(Dn, 1), 0.0).astype(np.float32)
    Binv = np.where(Bm > 0, 1.0 / np.maximum(Bm, 1), 0.0).astype(np.float32)

    def fold(Wh, att, heads):
        F = Wh.shape[1] // heads
        Whr = Wh.reshape(DM, heads, F)
        wx = np.einsum("dhf,hf->dh", Whr, att[0, :, :F]).astype(np.float32)
        we = np.einsum("dhf,hf->dh", Whr, att[0, :, F:]).astype(np.float32)
        return wx, we

    wx1, we1 = fold(Wh1, att1, 4)
    wx2, we2 = fold(Wh2, att2, 1)

    cpk = np.zeros((128, 64), np.float32)
    cpk[:, 0] = b1
    cpk[:, 1:33] = b3.reshape(T, DM).T
    for i, k in enumerate(("g1", "be1", "g2", "be2", "g3", "be3",
                           "g4", "be4")):
        cpk[:, 33 + i] = np.asarray(inputs[k], np.float32)
    cpk[:, 41:49] = (Binv / 4.0).reshape(MT, 128).T
    cpk[:, 49:57] = Binv.reshape(MT, 128).T

    wpk = np.zeros((128, 656), np.float32)
    wpk[:, 0:512] = Wh1
    wpk[:, 512:516] = wx1
    wpk[:, 516:520] = we1
    wpk[:, 520:648] = Wh2
    wpk[:, 648:649] = wx2
    wpk[:, 649:650] = we2

    shared = {
        "W1b": np.ascontiguousarray(
            W1.reshape(KT, 128, DM).transpose(1, 0, 2)).astype(bf),
        "W3b": np.ascontiguousarray(W3).astype(bf),
        "cpack": cpk,
        "wpack": wpk.astype(bf),
        "bmr": np.ascontiguousarray(Bm.reshape(1, M)),
    }

    in_maps = []
    for c in range(NCORES):
        rows = slice(c * NL, (c + 1) * NL)
        xT = np.ascontiguousarray(x[rows].reshape(NL, D_IN).T)  # [4096, NL]
        Sl = S[:, rows]                                          # [M, NL]
        S_nm_c = np.ascontiguousarray(
            Sl.T.reshape(NT, 128, M).transpose(1, 0, 2)).astype(bf)
        m = dict(shared)
        m["xTb"] = np.ascontiguousarray(
            xT.reshape(KT, 128, NL)).astype(bf)
        m["S_nm"] = S_nm_c
        m["dinvb"] = np.broadcast_to(
            Dinv[rows].reshape(1, NL), (128, NL)).astype(bf).copy()
        in_maps.append(m)
    return in_maps


def _run(inputs, trace=False, tmpdir=None):
    global _PROGRAM
    _ensure_ntff_hook()
    from concourse.bass_utils import run_bass_kernel_spmd

    if _PROGRAM is None:
        _PROGRAM = build_program()
    in_maps = _prep_inputs(inputs)
    res = run_bass_kernel_spmd(_PROGRAM, in_maps, list(range(NCORES)),
                               trace=trace, tmpdir=tmpdir)
    out = np.empty((N, T, DM), np.float32)
    for c in range(NCORES):
        oT = res.results[c]["outT"]                  # [4096, NL]
        out[c * NL:(c + 1) * NL] = oT.T.reshape(NL, T, DM)
    return out, res


def kernel(**inputs) -> np.ndarray:
    out, _ = _run(inputs)
    return out


if __name__ == "__main__":
    d = np.load("/root/problem/inputs.npz")
    inp = {k: d[k] for k in d.files}
    got = kernel(**inp)
    exp = np.load("/root/problem/expected.npy")
    denom = np.abs(exp).max()
    print("rel err:", np.abs(got - exp).max() / denom)


# revision 16
# speedup vs baseline: 2.0648x; 2.0648x over previous
"""HGAT block on 8 Trainium2 NeuronCores (Bass/Tile, SPMD node-sharded), v2.

Dense reformulation with rank-1 attention factorization:
  exp(lrelu(ax+ae)) = max(exp(ax)exp(ae), exp(ax/5)exp(ae/5))
so the dense per-head attention map Z[n,m] = S[n,m]*max(A1B1, A2B2) is built
in 3 elementwise passes from per-node columns (A) and broadcast per-edge
rows (B) -- no transcendentals over the dense map. The m-major copy of Z
(for the node-side aggregation matmul) comes from the XBAR DMA-transpose,
not PE transposes. Segment softmax is unnormalized: denominators den[m] ride
the per-head allreduce next to the eo partials, and Binv/den^2 (and the
1/heads mean) fold into the per-partition eoH scaling.

BatchNorms fold into weights: BN1/BN2 -> Wh', wsx', wse' + bias rows computed
on-device after a stats allreduce (stats packed into the he_attr allreduce);
BN3 -> W3', b3'; BN4 applied as a final affine after a small stats allreduce.
All hyperedge collectives run in bf16.

Sharding: nodes N=8192 split 1024/core; per-edge partials (he_attr, eo, den)
and BN stats are all-reduced across the 8 cores.
"""

import sys
import types

import numpy as np

N, T, DM = 8192, 32, 128
M, NNZ = 1024, 131072
EPS = 1e-5
SLOPE = 0.2
NCORES = 8
NL = N // NCORES          # 1024 local nodes per core
NT = NL // 128            # 8 node tiles
MT = M // 128             # 8 edge tiles
KT = (T * DM) // 128      # 32 k-tiles
D_IN = T * DM             # 4096
CW = 1032                 # per-head collective width: 1024 eo + 8 den

_PROGRAM = None


def _ensure_ntff_hook():
    try:
        import antenv.axon_hooks  # noqa: F401
        return
    except ImportError:
        pass
    try:
        import antenv
        from trn_agent_boot.trn_boot import _ntff_profile_via_ctypes
    except ImportError:
        return
    mod = types.ModuleType("antenv.axon_hooks")
    hook = _ntff_profile_via_ctypes("/opt/axon/libaxon_pjrt.so")
    mod.get_axon_ntff_profile_hook = lambda: hook
    mod.set_axon_ntff_profile_hook = lambda h: None
    sys.modules["antenv.axon_hooks"] = mod
    antenv.axon_hooks = mod


def build_program():
    from concourse import bacc, mybir, tile, masks

    f32 = mybir.dt.float32
    bf16 = mybir.dt.bfloat16
    AF = mybir.ActivationFunctionType
    ALU = mybir.AluOpType
    AX = mybir.AxisListType
    RG = [list(range(NCORES))]

    nc = bacc.Bacc("TRN2", target_bir_lowering=False, debug=False,
                   num_devices=NCORES)

    def din(name, shape, dt=f32):
        return nc.dram_tensor(name, list(shape), dt, kind="ExternalInput")

    xTb = din("xTb", [KT, 128, NL], bf16)
    S_nm = din("S_nm", [128, NT, M], bf16)
    W1b = din("W1b", [128, KT, DM], bf16)
    W3b = din("W3b", [DM, D_IN], bf16)
    cpack = din("cpack", [128, 64], f32)
    wpack = din("wpack", [128, 656], bf16)
    bmr = din("bmr", [1, M], f32)
    dinvb = din("dinvb", [128, NL], bf16)
    outT = nc.dram_tensor("outT", [D_IN, NL], f32, kind="ExternalOutput")

    def cc_pair(name, shape, dt):
        i = nc.dram_tensor(f"{name}_in", list(shape), dt)
        o = nc.dram_tensor(f"{name}_out", list(shape), dt,
                           addr_space="Shared")
        return i, o

    he1_in, he1_out = cc_pair("he1", [128, M + 2], bf16)
    he2_in, he2_out = cc_pair("he2", [128, M + 2], bf16)
    eo1a_in, eo1a_out = cc_pair("eo1a", [128, 2 * CW], bf16)
    eo1b_in, eo1b_out = cc_pair("eo1b", [128, 2 * CW], bf16)
    eo2_in, eo2_out = cc_pair("eo2", [128, CW], bf16)
    bn3_in, bn3_out = cc_pair("bn3", [128, 2], f32)
    bn4_in, bn4_out = cc_pair("bn4", [128, 2], f32)

    # cpack columns
    C_B1, C_B3, C_G, C_BINV4, C_BINV1 = 0, 1, 33, 41, 49
    # wpack columns
    W_WH1, W_WX1, W_WE1, W_WH2, W_WX2, W_WE2 = 0, 512, 516, 520, 648, 649

    def allreduce(dst, src):
        nc.gpsimd.collective_compute(
            "AllReduce", ALU.add, replica_groups=RG,
            ins=[src[:].opt()], outs=[dst[:].opt()])

    with tile.TileContext(nc) as tc:
        with (
            tc.tile_pool(name="const", bufs=1) as constp,
            tc.tile_pool(name="state", bufs=1) as statep,
            tc.tile_pool(name="bbc", bufs=4) as bbcp,
            tc.tile_pool(name="zn", bufs=2) as znp,
            tc.tile_pool(name="zm", bufs=3) as zmp,
            tc.tile_pool(name="xk", bufs=3) as xkp,
            tc.tile_pool(name="work", bufs=4) as workp,
            tc.tile_pool(name="ps_no", bufs=1, space="PSUM") as ps_no_p,
            tc.tile_pool(name="ps_eo", bufs=2, space="PSUM") as ps_eo_p,
            tc.tile_pool(name="ps_sm", bufs=2, space="PSUM") as ps_sm_p,
        ):
            ident = constp.tile([128, 128], bf16)
            masks.make_identity(nc, ident[:])
            ones_b = constp.tile([1, 128], bf16, tag="ones_b")
            nc.gpsimd.memset(ones_b[:], 1.0)
            ones_f = constp.tile([1, 128], f32, tag="ones_f")
            nc.gpsimd.memset(ones_f[:], 1.0)
            eps_sb = constp.tile([128, 1], f32, tag="epsc")
            nc.gpsimd.memset(eps_sb[:], EPS)

            cp = constp.tile([128, 64], f32, tag="cpack")
            nc.gpsimd.dma_start(cp[:], cpack[:])
            wp = constp.tile([128, 656], bf16, tag="wpack")
            nc.gpsimd.dma_start(wp[:], wpack[:])
            bm_sb = constp.tile([1, M], f32, tag="bmr")
            nc.gpsimd.dma_start(bm_sb[:], bmr[:])
            dinv_bc = constp.tile([128, NL], bf16, tag="dinvb")
            nc.gpsimd.dma_start(dinv_bc[:], dinvb[:])
            s_sb = constp.tile([128, NT * M], bf16, tag="snm")
            nc.gpsimd.dma_start(s_sb[:], S_nm[:].rearrange("p n m -> p (n m)"))
            w_sb = constp.tile([128, KT * DM], bf16, tag="wslot", name="w1sb")
            nc.gpsimd.dma_start(w_sb[:], W1b[:].rearrange("p k d -> p (k d)"))

            def s_tile(nt):
                return s_sb[:, nt * M:(nt + 1) * M]

            def gb(i):
                return cp[:, C_G + i:C_G + i + 1]

            # ---- helpers -------------------------------------------------
            def transpose_cols(src_fn, dst, n128, dt=bf16):
                for q in range(0, n128, 4):
                    w = min(4, n128 - q)
                    trq = ps_sm_p.tile([128, 512], dt, tag="sm", name="trq")
                    for k in range(w):
                        nc.tensor.matmul(trq[:, k * 128:(k + 1) * 128],
                                         src_fn(q + k), ident[:],
                                         is_transpose=True)
                    nc.vector.tensor_copy(
                        dst[:, q * 128:(q + w) * 128], trq[:, 0:w * 128])

            dump = statep.tile([128, NL], bf16, tag="dump")

            def bn_scales(sum_ap, sumsq_ap, g_ap, be_ap, count, tagp):
                sc = workp.tile([128, 1], f32, tag=f"sc{tagp}",
                                name=f"sc{tagp}", bufs=1)
                sh = workp.tile([128, 1], f32, tag=f"sh{tagp}",
                                name=f"sh{tagp}", bufs=1)
                tmp = workp.tile([128, 4], f32, tag="bnt", name=f"bnt{tagp}")
                mean, var, m2, rstd = (tmp[:, i:i + 1] for i in range(4))
                nc.scalar.mul(mean, sum_ap, 1.0 / count)
                nc.scalar.mul(var, sumsq_ap, 1.0 / count)
                nc.scalar.square(m2, mean)
                nc.vector.tensor_sub(var, var, m2)
                nc.scalar.activation(rstd, var, AF.Sqrt, bias=eps_sb[:, 0:1])
                nc.vector.reciprocal(rstd, rstd)
                nc.vector.tensor_mul(sc, g_ap, rstd)
                nc.vector.tensor_mul(sh, mean, sc)
                nc.vector.tensor_sub(sh, be_ap, sh)
                return sc, sh

            # ================= W1: h1 = lrelu(x@W1 + b1) ==================
            hT = statep.tile([128, NL], f32, tag="hT")
            hT_b = statep.tile([128, NL], bf16, tag="hTb")
            hp = ps_eo_p.tile([128, 1024], f32, tag="eo", name="w1p")
            for kt in range(KT):
                xk = xkp.tile([128, NL], bf16, tag="xk")
                nc.gpsimd.dma_start(xk[:], xTb[kt, :, :])
                for i in range(2):
                    nc.tensor.matmul(
                        hp[:, i * 512:(i + 1) * 512],
                        w_sb[:, kt * DM:(kt + 1) * DM],
                        xk[:, i * 512:(i + 1) * 512],
                        start=(kt == 0), stop=(kt == KT - 1))
            for i in range(2):
                sl = slice(i * 512, (i + 1) * 512)
                nc.scalar.activation(hT[:, sl], hp[:, sl], AF.Prelu,
                                     bias=cp[:, C_B1:C_B1 + 1], alpha=SLOPE)
            nc.vector.tensor_copy(hT_b[:], hT[:])
            # W3 load into the W slot (free after last W1 matmul)
            w3_sb = constp.tile([128, KT * DM], bf16, tag="wslot",
                                name="w3sb")
            nc.gpsimd.dma_start(w3_sb[:], W3b[:])

            # ============ per-layer bn+he edge (stats + he matmul) ========
            def bn_he_edge(lname, he_in, he_out):
                st = workp.tile([128, 2], f32, tag="st", name=f"st{lname}")
                nc.vector.reduce_sum(st[:, 0:1], hT[:], axis=AX.X)
                nc.scalar.activation(dump[:], hT[:], AF.Square,
                                     accum_out=st[:, 1:2])
                hn_b = statep.tile([128, NT * 128], bf16, tag="hnb",
                                   name=f"hnb{lname}")
                transpose_cols(
                    lambda i: hT_b[:, i * 128:(i + 1) * 128], hn_b[:], NT)
                hep = ps_eo_p.tile([128, 1024], f32, tag="eo",
                                   name=f"hep{lname}")
                for nt in range(NT):
                    for i in range(2):
                        nc.tensor.matmul(
                            hep[:, i * 512:(i + 1) * 512],
                            hn_b[:, nt * 128:(nt + 1) * 128],
                            s_tile(nt)[:, i * 512:(i + 1) * 512],
                            start=(nt == 0), stop=(nt == NT - 1))
                hei = workp.tile([128, M + 2], bf16, tag="hei",
                                 name=f"hei{lname}", bufs=1)
                nc.scalar.copy(hei[:, 0:M], hep[:])
                nc.vector.tensor_copy(hei[:, M:M + 2], st[:])
                nc.gpsimd.dma_start(he_in[:], hei[:])
                allreduce(he_out, he_in)

            # ============ per-layer post-AR folds =========================
            def bn_folds(lname, he_out, gi, heads, wh_sl, wx_sl, we_sl):
                HF = heads * DM
                er = statep.tile([128, M + 2], bf16, tag="her",
                                 name=f"her{lname}")
                nc.gpsimd.dma_start(er[:], he_out[:])
                strf = workp.tile([128, 2], f32, tag="st",
                                  name=f"strf{lname}")
                nc.vector.tensor_copy(strf[:], er[:, M:M + 2])
                sc, sh = bn_scales(strf[:, 0:1], strf[:, 1:2],
                                   gb(gi), gb(gi + 1), float(N), lname)
                nc.scalar.activation(hT[:], hT[:], AF.Prelu, bias=sh[:, 0:1],
                                     scale=sc[:, 0:1], alpha=1.0)
                nc.vector.tensor_copy(hT_b[:], hT[:])
                whp = statep.tile([128, 512], bf16, tag="whp",
                                  name=f"whp{lname}")
                nc.vector.tensor_scalar(whp[:, 0:HF], wp[:, wh_sl],
                                        sc[:, 0:1], None, op0=ALU.mult)
                wxp = statep.tile([128, 4], bf16, tag="wxp",
                                  name=f"wxp{lname}")
                nc.vector.tensor_scalar(wxp[:, 0:heads], wp[:, wx_sl],
                                        sc[:, 0:1], None, op0=ALU.mult)
                wsep = statep.tile([128, 4], bf16, tag="wsep",
                                   name=f"wsep{lname}")
                nc.vector.tensor_scalar(wsep[:, 0:heads], wp[:, we_sl],
                                        sc[:, 0:1], None, op0=ALU.mult)
                sh_b = workp.tile([128, 1], bf16, tag="shb",
                                  name=f"shb{lname}")
                nc.vector.tensor_copy(sh_b[:], sh[:])
                # bias_f row = sh^T Wh ; axc row = sh^T wsx  (raw weights)
                bps = ps_sm_p.tile([128, 512], f32, tag="sm", name="bps")
                nc.tensor.matmul(bps[0:1, 0:HF], sh_b[:], wp[:, wh_sl],
                                 start=True, stop=True)
                # axc row = sh^T wsx ; aec row = sh^T wse
                aps = ps_sm_p.tile([128, 512], f32, tag="sm", name="aps")
                nc.tensor.matmul(aps[0:1, 8:8 + heads], sh_b[:],
                                 wp[:, wx_sl], start=True, stop=True)
                nc.tensor.matmul(aps[0:1, 16:16 + heads], sh_b[:],
                                 wp[:, we_sl], start=True, stop=True)
                brow = statep.tile([1, 528], f32, tag="brow",
                                   name=f"brow{lname}")
                nc.vector.tensor_copy(brow[0:1, 0:HF], bps[0:1, 0:HF])
                nc.vector.tensor_copy(brow[0:1, 512:512 + heads],
                                      aps[0:1, 8:8 + heads])
                nc.vector.tensor_copy(brow[0:1, 520:520 + heads],
                                      aps[0:1, 16:16 + heads])
                axcb = statep.tile([128, 8], f32, tag="axcb",
                                   name=f"axcb{lname}")
                nc.gpsimd.partition_broadcast(axcb[:, 0:heads],
                                              brow[0:1, 512:512 + heads])
                nc.vector.tensor_scalar(axcb[:, 4:4 + heads],
                                        axcb[:, 0:heads], 0.2, None,
                                        op0=ALU.mult)
                bbps = ps_sm_p.tile([128, 512], f32, tag="sm", name="bbps")
                nc.tensor.matmul(bbps[:, 0:HF], ones_f[:],
                                 brow[0:1, 0:HF], start=True, stop=True)
                bias_bc = statep.tile([128, 512], f32, tag="biasbc",
                                      name=f"bbc{lname}")
                nc.vector.tensor_copy(bias_bc[:, 0:HF], bbps[:, 0:HF])
                return er, whp, wxp, wsep, brow, axcb, bias_bc

            # ============ layer compute: xt/ax + A/B prep =================
            def layer_prep(lname, heads, er, whp, wxp, wsep, brow, axcb,
                           bias_bc):
                HF = heads * DM
                xta = statep.tile([128, NT * 512], bf16, tag="xta",
                                  name=f"xta{lname}")
                axn = statep.tile([128, 8 * NT], f32, tag="axn",
                                  name=f"axn{lname}")
                axn3 = axn[:].rearrange("p (h t) -> p h t", h=8)
                for nt in range(NT):
                    xps = ps_eo_p.tile([128, 1024], f32, tag="eo",
                                       name="xps")
                    axps = ps_sm_p.tile([128, 512], f32, tag="sm",
                                        name="axps")
                    nc.tensor.matmul(xps[:, 0:HF],
                                     hT_b[:, nt * 128:(nt + 1) * 128],
                                     whp[:, 0:HF], start=True, stop=True)
                    nc.tensor.matmul(axps[:, 0:heads],
                                     hT_b[:, nt * 128:(nt + 1) * 128],
                                     wxp[:, 0:heads], start=True, stop=True)
                    nc.vector.scalar_tensor_tensor(
                        xta[:, nt * 512:nt * 512 + HF], xps[:, 0:HF], 1.0,
                        bias_bc[:, 0:HF], op0=ALU.mult, op1=ALU.add)
                    nc.vector.tensor_copy(axn3[:, 0:heads, nt],
                                          axps[:, 0:heads])
                A1 = statep.tile([128, 8 * NT], f32, tag="A1",
                                 name=f"A1{lname}")
                A2 = statep.tile([128, 8 * NT], f32, tag="A2",
                                 name=f"A2{lname}")
                for h in range(heads):
                    sl = slice(h * NT, (h + 1) * NT)
                    nc.scalar.activation(A1[:, sl], axn[:, sl], AF.Exp,
                                         bias=axcb[:, h:h + 1])
                    nc.scalar.activation(A2[:, sl], axn[:, sl], AF.Exp,
                                         scale=0.2, bias=axcb[:, 4 + h:5 + h])
                ae_rows = []
                for h in range(heads):
                    aeps = ps_sm_p.tile([128, 512], f32, tag="sm",
                                        name="aeps")
                    aeps2 = ps_sm_p.tile([128, 512], f32, tag="sm",
                                         name="aeps2")
                    nc.tensor.matmul(aeps[0:1, :], wsep[:, h:h + 1],
                                     er[:, 0:512], start=True, stop=True)
                    nc.tensor.matmul(aeps2[0:1, :], wsep[:, h:h + 1],
                                     er[:, 512:M], start=True, stop=True)
                    aer = statep.tile([1, M], f32, tag=f"aer{h}",
                                      name=f"aer{lname}{h}")
                    aecs = brow[0:1, 520 + h:521 + h]
                    nc.vector.scalar_tensor_tensor(
                        aer[0:1, 0:512], bm_sb[0:1, 0:512], aecs,
                        aeps[0:1, :], op0=ALU.mult, op1=ALU.add)
                    nc.vector.scalar_tensor_tensor(
                        aer[0:1, 512:M], bm_sb[0:1, 512:M], aecs,
                        aeps2[0:1, :], op0=ALU.mult, op1=ALU.add)
                    ae_rows.append(aer)
                return xta, A1, A2, ae_rows

            # ============ per-head map build + eoT + den ==================
            def head_maps(lname, h, hg, xta, A1, A2, ae_rows, ar_sb):
                b1bc = bbcp.tile([128, M], bf16, tag="bbc",
                                 name=f"b1bc{lname}{h}")
                b2bc = bbcp.tile([128, M], bf16, tag="bbc",
                                 name=f"b2bc{lname}{h}")
                bps = ps_sm_p.tile([128, 512], f32, tag="sm", name="bcp")
                bps2 = ps_sm_p.tile([128, 512], f32, tag="sm", name="bcp2")
                nc.tensor.matmul(bps[:], ones_f[:], ae_rows[h][0:1, 0:512],
                                 start=True, stop=True)
                nc.tensor.matmul(bps2[:], ones_f[:], ae_rows[h][0:1, 512:M],
                                 start=True, stop=True)
                nc.scalar.activation(b1bc[:, 0:512], bps[:], AF.Exp)
                nc.scalar.activation(b1bc[:, 512:M], bps2[:], AF.Exp)
                nc.scalar.activation(b2bc[:, 0:512], bps[:], AF.Exp,
                                     scale=0.2)
                nc.scalar.activation(b2bc[:, 512:M], bps2[:], AF.Exp,
                                     scale=0.2)
                znm = znp.tile([128, NT * M], bf16, tag="znm",
                               name=f"znm{lname}{h}")
                zmn = zmp.tile([128, MT * NL], bf16, tag="zmn",
                               name=f"zmn{lname}{h}")
                zmn3 = zmn[:].rearrange("p (mt n) -> p mt n", mt=MT)
                eop = ps_eo_p.tile([128, 1024], f32, tag="eo",
                                   name=f"eop{lname}{h}")
                for nt in range(NT):
                    zsl = znm[:, nt * M:(nt + 1) * M]
                    u = workp.tile([128, M], bf16, tag="u", name="u", bufs=2)
                    nc.vector.tensor_scalar(
                        u[:], b2bc[:], A2[:, h * NT + nt:h * NT + nt + 1],
                        None, op0=ALU.mult)
                    nc.vector.scalar_tensor_tensor(
                        zsl, b1bc[:], A1[:, h * NT + nt:h * NT + nt + 1],
                        u[:], op0=ALU.mult, op1=ALU.max)
                    nc.vector.tensor_mul(zsl, zsl, s_tile(nt))
                    xsl = xta[:, nt * 512 + h * DM:nt * 512 + (h + 1) * DM]
                    for i in range(2):
                        nc.tensor.matmul(
                            eop[:, i * 512:(i + 1) * 512], xsl,
                            zsl[:, i * 512:(i + 1) * 512],
                            start=(nt == 0), stop=(nt == NT - 1))
                    nc.sync.dma_start_transpose(
                        zmn3[:, :, nt * 128:(nt + 1) * 128], zsl)
                off = hg * CW
                with nc.allow_low_precision(
                        reason="den ~O(100), bf16 rounding ~0.4% ok"):
                    for mt in range(MT):
                        nc.vector.reduce_sum(ar_sb[:, off + 1024 + mt:
                                                   off + 1024 + mt + 1],
                                             zmn[:, mt * NL:(mt + 1) * NL],
                                             axis=AX.X)
                nc.scalar.copy(ar_sb[:, off:off + 1024], eop[:])
                return znm, zmn

            # ============ per-head post-AR: eoH + noT =====================
            def head_no(lname, h, hg, er_eo, zmn, nop, first, last,
                        binv_col):
                off = hg * CW
                sm = workp.tile([128, MT], f32, tag="smh",
                                name=f"sm{lname}{h}")
                nc.vector.tensor_scalar(sm[:], er_eo[:, off + 1024:off + CW],
                                        1e-30, None, op0=ALU.max)
                nc.vector.reciprocal(sm[:], sm[:])
                nc.vector.tensor_mul(sm[:], sm[:], sm[:])
                nc.vector.tensor_mul(sm[:], sm[:], binv_col)
                eoh = workp.tile([128, MT * 128], bf16, tag="eoh",
                                 name=f"eoh{lname}{h}", bufs=2)
                nc.scalar.dma_start_transpose(
                    eoh[:].rearrange("p (mt c) -> p mt c", mt=MT),
                    er_eo[:, off:off + 1024])
                for mt in range(MT):
                    esl = eoh[:, mt * 128:(mt + 1) * 128]
                    nc.vector.tensor_scalar(esl, esl, sm[:, mt:mt + 1],
                                            None, op0=ALU.mult)
                    for i in range(2):
                        nc.tensor.matmul(
                            nop[:, i * 512:(i + 1) * 512], esl,
                            zmn[:, mt * NL + i * 512:mt * NL + (i + 1) * 512],
                            start=(first and mt == 0),
                            stop=(last and mt == MT - 1))

            def epilogue(lname, nop):
                nsc = workp.tile([128, NL], f32, tag="nsc",
                                 name=f"nsc{lname}", bufs=1)
                nc.vector.tensor_mul(nsc[:], nop[:], dinv_bc[:])
                nc.vector.tensor_add(hT[:], hT[:], nsc[:])
                nc.vector.tensor_copy(hT_b[:], hT[:])

            # ================== LAYER 1 (4 heads) =========================
            bn_he_edge("1", he1_in, he1_out)
            er1, whp1, wxp1, wsep1, aec1, axcb1, bias_bc1 = \
                bn_folds("1", he1_out, 0, 4,
                         slice(W_WH1, W_WH1 + 512),
                         slice(W_WX1, W_WX1 + 4), slice(W_WE1, W_WE1 + 4))
            xta1, A1_1, A2_1, ae1 = layer_prep("1", 4, er1, whp1, wxp1,
                                               wsep1, aec1, axcb1, bias_bc1)
            nop1 = ps_no_p.tile([128, NL], f32, tag="no", name="no1")
            binv4 = cp[:, C_BINV4:C_BINV4 + MT]

            ar1a = statep.tile([128, 2 * CW], bf16, tag="ar1a")
            zA = [head_maps("1", h, h, xta1, A1_1, A2_1, ae1, ar1a)
                  for h in range(2)]
            nc.gpsimd.dma_start(eo1a_in[:], ar1a[:])
            allreduce(eo1a_out, eo1a_in)

            ar1b = statep.tile([128, 2 * CW], bf16, tag="ar1b")
            zB = [head_maps("1", h, h - 2, xta1, A1_1, A2_1, ae1, ar1b)
                  for h in range(2, 4)]
            nc.gpsimd.dma_start(eo1b_in[:], ar1b[:])
            allreduce(eo1b_out, eo1b_in)

            er1a = statep.tile([128, 2 * CW], bf16, tag="ar1a", name="er1a")
            nc.gpsimd.dma_start(er1a[:], eo1a_out[:])
            for h in range(2):
                head_no("1", h, h, er1a, zA[h][1], nop1,
                        first=(h == 0), last=False, binv_col=binv4)
            er1b = statep.tile([128, 2 * CW], bf16, tag="ar1b", name="er1b")
            nc.gpsimd.dma_start(er1b[:], eo1b_out[:])
            for h in range(2, 4):
                head_no("1", h, h - 2, er1b, zB[h - 2][1], nop1,
                        first=False, last=(h == 3), binv_col=binv4)
            epilogue("1", nop1)

            # ================== LAYER 2 (1 head) ==========================
            bn_he_edge("2", he2_in, he2_out)
            er2, whp2, wxp2, wsep2, aec2, axcb2, bias_bc2 = \
                bn_folds("2", he2_out, 2, 1,
                         slice(W_WH2, W_WH2 + 128),
                         slice(W_WX2, W_WX2 + 1), slice(W_WE2, W_WE2 + 1))
            xta2, A1_2, A2_2, ae2 = layer_prep("2", 1, er2, whp2, wxp2,
                                               wsep2, aec2, axcb2, bias_bc2)
            nop2 = ps_no_p.tile([128, NL], f32, tag="no", name="no2")
            binv1 = cp[:, C_BINV1:C_BINV1 + MT]

            ar2 = statep.tile([128, CW], bf16, tag="ar2")
            z2 = head_maps("2", 0, 0, xta2, A1_2, A2_2, ae2, ar2)
            nc.gpsimd.dma_start(eo2_in[:], ar2[:])
            allreduce(eo2_out, eo2_in)
            er2e = statep.tile([128, CW], bf16, tag="ar2", name="er2e")
            nc.gpsimd.dma_start(er2e[:], eo2_out[:])
            head_no("2", 0, 0, er2e, z2[1], nop2,
                    first=True, last=True, binv_col=binv1)
            epilogue("2", nop2)

            # ================== BN3 (stats only) ==========================
            st3 = workp.tile([128, 2], f32, tag="st", name="st3")
            nc.vector.reduce_sum(st3[:, 0:1], hT[:], axis=AX.X)
            nc.scalar.activation(dump[:], hT[:], AF.Square,
                                 accum_out=st3[:, 1:2])
            nc.gpsimd.dma_start(bn3_in[:], st3[:])
            allreduce(bn3_out, bn3_in)
            st3r = workp.tile([128, 2], f32, tag="st", name="st3r")
            nc.gpsimd.dma_start(st3r[:], bn3_out[:])
            sc3, sh3 = bn_scales(st3r[:, 0:1], st3r[:, 1:2],
                                 gb(4), gb(5), float(N), "3")
            nc.vector.tensor_scalar(w3_sb[:], w3_sb[:], sc3[:, 0:1],
                                    None, op0=ALU.mult)
            shr = workp.tile([128, 2], f32, tag="shr", name="shr", bufs=1)
            nc.vector.reciprocal(shr[:, 0:1], sc3[:, 0:1])
            nc.vector.tensor_mul(shr[:, 0:1], shr[:, 0:1], sh3[:, 0:1])
            shr_b = workp.tile([128, 1], bf16, tag="shrb", name="shrb",
                               bufs=1)
            nc.vector.tensor_copy(shr_b[:], shr[:, 0:1])

            # ============ W3 pass A: v = x + lrelu(h3@W3' + b3') ==========
            vch = [znp.tile([128, NT * M], bf16, tag="znm", name=f"vch{i}")
                   for i in range(2)] + \
                  [zmp.tile([128, MT * NL], bf16, tag="zmn", name=f"vch{i}")
                   for i in range(2, 4)]

            def v_slice(jc):
                t = vch[jc // 8]
                j = jc % 8
                return t[:, j * NL:(j + 1) * NL]

            vsum = statep.tile([128, KT], f32, tag="vsum")
            vsq = statep.tile([128, KT], f32, tag="vsq")
            for jc in range(KT):
                yps = ps_eo_p.tile([128, 1024], f32, tag="eo", name="yps")
                cps = ps_sm_p.tile([128, 512], f32, tag="sm", name="cps")
                wsl = w3_sb[:, jc * 128:(jc + 1) * 128]
                for i in range(2):
                    nc.tensor.matmul(yps[:, i * 512:(i + 1) * 512], wsl,
                                     hT_b[:, i * 512:(i + 1) * 512],
                                     start=True, stop=True)
                nc.tensor.matmul(cps[:, 0:1], wsl, shr_b[:],
                                 start=True, stop=True)
                bias_sb = workp.tile([128, 1], f32, tag="b3s", name="b3s",
                                     bufs=2)
                nc.vector.tensor_add(bias_sb[:], cps[:, 0:1],
                                     cp[:, C_B3 + jc:C_B3 + jc + 1])
                vtmp = workp.tile([128, NL], bf16, tag="vtmp", name="vtmp",
                                  bufs=2)
                for i in range(2):
                    sl = slice(i * 512, (i + 1) * 512)
                    nc.scalar.activation(vtmp[:, sl], yps[:, sl], AF.Prelu,
                                         bias=bias_sb[:, 0:1], alpha=SLOPE)
                xres = xkp.tile([128, NL], bf16, tag="xk", name="xres")
                nc.gpsimd.dma_start(xres[:], xTb[jc, :, :])
                vd = v_slice(jc)
                nc.vector.scalar_tensor_tensor(
                    vd, vtmp[:], 1.0, xres[:], op0=ALU.mult, op1=ALU.add,
                    accum_out=vsum[:, jc:jc + 1])
                nc.scalar.activation(dump[:], vd, AF.Square,
                                     accum_out=vsq[:, jc:jc + 1])
            st4 = workp.tile([128, 2], f32, tag="st4", name="st4", bufs=1)
            nc.vector.reduce_sum(st4[:, 0:1], vsum[:], axis=AX.X)
            nc.vector.reduce_sum(st4[:, 1:2], vsq[:], axis=AX.X)
            nc.gpsimd.dma_start(bn4_in[:], st4[:])
            allreduce(bn4_out, bn4_in)
            st4r = workp.tile([128, 2], f32, tag="st", name="st4r")
            nc.gpsimd.dma_start(st4r[:], bn4_out[:])
            sc4, sh4 = bn_scales(st4r[:, 0:1], st4r[:, 1:2],
                                 gb(6), gb(7), float(N * T), "4")

            # ============ W3 pass B: out = sc4*v + sh4 ====================
            for jc in range(KT):
                ot = workp.tile([128, NL], f32, tag="ot", name="ot", bufs=2)
                vd = v_slice(jc)
                if jc % 2 == 0:
                    nc.scalar.activation(ot[:], vd, AF.Prelu,
                                         bias=sh4[:, 0:1],
                                         scale=sc4[:, 0:1], alpha=1.0)
                else:
                    nc.vector.tensor_scalar(ot[:], vd, sc4[:, 0:1],
                                            sh4[:, 0:1],
                                            op0=ALU.mult, op1=ALU.add)
                nc.gpsimd.dma_start(outT[jc * 128:(jc + 1) * 128, :], ot[:])

    nc.compile()
    return nc


def _prep_inputs(inputs):
    """Host-side preprocessing: shard, transpose, fold weights, build S."""
    x = np.ascontiguousarray(np.asarray(inputs["x"], np.float32))
    he_n = np.asarray(inputs["he_nodes"]).astype(np.int64)
    he_e = np.asarray(inputs["he_edges"]).astype(np.int64)
    W1 = np.asarray(inputs["W1"], np.float32)
    b1 = np.asarray(inputs["b1"], np.float32)
    Wh1 = np.asarray(inputs["Wh1"], np.float32)
    att1 = np.asarray(inputs["att1"], np.float32)
    Wh2 = np.asarray(inputs["Wh2"], np.float32)
    att2 = np.asarray(inputs["att2"], np.float32)
    W3 = np.asarray(inputs["W3"], np.float32)
    b3 = np.asarray(inputs["b3"], np.float32)

    try:
        import ml_dtypes
        bf = ml_dtypes.bfloat16
    except ImportError:  # pragma: no cover
        import jax.numpy as jnp
        bf = jnp.bfloat16

    S = np.zeros((M, N), np.float32)
    np.add.at(S, (he_e, he_n), 1.0)
    Dn = S.sum(axis=0)
    Bm = S.sum(axis=1)
    Dinv = np.where(Dn > 0, 1.0 / np.maximum(Dn, 1), 0.0).astype(np.float32)
    Binv = np.where(Bm > 0, 1.0 / np.maximum(Bm, 1), 0.0).astype(np.float32)

    def fold(Wh, att, heads):
        F = Wh.shape[1] // heads
        Whr = Wh.reshape(DM, heads, F)
        wx = np.einsum("dhf,hf->dh", Whr, att[0, :, :F]).astype(np.float32)
        we = np.einsum("dhf,hf->dh", Whr, att[0, :, F:]).astype(np.float32)
        return wx, we

    wx1, we1 = fold(Wh1, att1, 4)
    wx2, we2 = fold(Wh2, att2, 1)

    cpk = np.zeros((128, 64), np.float32)
    cpk[:, 0] = b1
    cpk[:, 1:33] = b3.reshape(T, DM).T
    for i, k in enumerate(("g1", "be1", "g2", "be2", "g3", "be3",
                           "g4", "be4")):
        cpk[:, 33 + i] = np.asarray(inputs[k], np.float32)
    cpk[:, 41:49] = (Binv / 4.0).reshape(MT, 128).T
    cpk[:, 49:57] = Binv.reshape(MT, 128).T

    wpk = np.zeros((128, 656), np.float32)
    wpk[:, 0:512] = Wh1
    wpk[:, 512:516] = wx1
    wpk[:, 516:520] = we1
    wpk[:, 520:648] = Wh2
    wpk[:, 648:649] = wx2
    wpk[:, 649:650] = we2

    shared = {
        "W1b": np.ascontiguousarray(
            W1.reshape(KT, 128, DM).transpose(1, 0, 2)).astype(bf),
        "W3b": np.ascontiguousarray(W3).astype(bf),
        "cpack": cpk,
        "wpack": wpk.astype(bf),
        "bmr": np.ascontiguousarray(Bm.reshape(1, M)),
    }

    in_maps = []
    for c in range(NCORES):
        rows = slice(c * NL, (c + 1) * NL)
        xT = np.ascontiguousarray(x[rows].reshape(NL, D_IN).T)  # [4096, NL]
        Sl = S[:, rows]                                          # [M, NL]
        S_nm_c = np.ascontiguousarray(
            Sl.T.reshape(NT, 128, M).transpose(1, 0, 2)).astype(bf)
        m = dict(shared)
        m["xTb"] = np.ascontiguousarray(
            xT.reshape(KT, 128, NL)).astype(bf)
        m["S_nm"] = S_nm_c
        m["dinvb"] = np.broadcast_to(
            Dinv[rows].reshape(1, NL), (128, NL)).astype(bf).copy()
        in_maps.append(m)
    return in_maps


def _run(inputs, trace=False, tmpdir=None):
    global _PROGRAM
    _ensure_ntff_hook()
    from concourse.bass_utils import run_bass_kernel_spmd

    if _PROGRAM is None:
        _PROGRAM = build_program()
    in_maps = _prep_inputs(inputs)
    res = run_bass_kernel_spmd(_PROGRAM, in_maps, list(range(NCORES)),
                               trace=trace, tmpdir=tmpdir)
    out = np.empty((N, T, DM), np.float32)
    for c in range(NCORES):
        oT = res.results[c]["outT"]                  # [4096, NL]
        out[c * NL:(c + 1) * NL] = oT.T.reshape(NL, T, DM)
    return out, res


def kernel(**inputs) -> np.ndarray:
    out, _ = _run(inputs)
    return out


if __name__ == "__main__":
    d = np.load("/root/problem/inputs.npz")
    inp = {k: d[k] for k in d.files}
    got = kernel(**inp)
    exp = np.load("/root/problem/expected.npy")
    denom = np.abs(exp).max()
    print("rel err:", np.abs(got - exp).max() / denom)
